# revision 17
# baseline (speedup 1.0000x reference)
"""Trainium2 Bass kernel for the soft-DTW cosine-distance loss.

Math
----
reference loss[b] = sdtw(TGT_b, X_b) - 0.5*sdtw(TGT_b,TGT_b)
                  - sdtw(OTH_b, X_b) + 0.5*sdtw(OTH_b,OTH_b)
(the shared sdtw(X,X) terms cancel), where sdtw is soft-DTW (gamma=1) on
the cosine-distance matrix D = 1 - cossim.

Device algorithm (per problem = one (pair, batch) cost matrix):
 * Gram matrices G = x_hat @ y_hat^T via TensorE in fp16 (row-normalized
   operands), E = exp(G - 1) = exp(-D) via ScalarE, stored fp16.
 * Soft-DTW in exp space: S[i,j] = sum over monotone paths of prod E.
   Column-major linear recurrence
       s_j[i] = (s_{j-1}[i] + s_{j-1}[i-1] + s_j[i-1]) * E[i,j]
   maps onto a single fused VectorE `tensor_tensor_scan`
       state = (t1[i] + state) * E[i]   with t1 = s_{j-1} + shift(s_{j-1})
   i.e. 2 DVE ops per DP column.  R = -log S[N-1,M-1].
 * The 384 lanes are split into NQ=4 blocks of NL=96 so all 32 problems
   x 4 blocks fill the 128 SBUF partitions; blocks advance in a wavefront
   (block q handles column j at wave w = q + j).  Block-crossing values
   (scan initial + shifted-neighbor edge) move between partition rows
   via tiny per-wave copies.
 * exp-space magnitude drift is compensated by multiplying the state by
   a fixed per-problem-type power of two every KC waves (pair-type drift
   rates are data-independent for this distribution); the exact total
   scale is added back on the host: R = -ln(s) - nev*KC*c*ln2.

Sharding: pure data parallel, batch dim 64 -> 8 cores x 8 batches.
"""

import copy as _copy
import math
import os

import numpy as np

import concourse.bass as bass
import concourse.tile as tile
from concourse import mybir

F32 = mybir.dt.float32
F16 = mybir.dt.float16
AF = mybir.ActivationFunctionType
ALU = mybir.AluOpType

# problem geometry (hardcoded per spec)
B, T, D = 64, 384, 512
NCORES = 8
BS = B // NCORES          # batches per core
NPAIR = 4                 # TX, TT, OX, OO
NP = NPAIR * BS           # problems per core
C_TYPES = (0.2, 1.0, 0.2, 1.0)   # log2-drift/column compensation per pair type


def _legalize_waits(nc, cap=1):
    """Split instructions carrying >cap sync waits (Tile's tail drain does;
    walrus codegen rejects them) into same-engine wait-carrying NoOps."""
    m = nc.m
    ctr = 0
    new_fns = []
    for fn in m.functions:
        new_fn = _copy.replace(fn, blocks=[])
        new_fn.set_allocations_from_list(fn.allocations)
        for blk in fn.blocks:
            out = []
            for inst in blk.instructions:
                si = inst.sync_info
                waits = list(si.on_wait) if si is not None else []
                if len(waits) > cap:
                    excess, keep = waits[:-cap], waits[-cap:]
                    for i in range(0, len(excess), cap):
                        nop = mybir.InstNoOp(
                            name=f"waitfix-{ctr}", engine=inst.engine
                        )
                        ctr += 1
                        nop.sync_info = mybir.SyncInfo(
                            on_wait=excess[i : i + cap], on_update=[]
                        )
                        out.append(nop)
                    si.on_wait = keep
                out.append(inst)
            new_fn.blocks.append(_copy.replace(blk, instructions=out))
        new_fns.append(new_fn)
    nc.m = _copy.replace(m, functions=new_fns)


def build_nc(BSc=BS, Tc=T, Dc=D, NQ=4, KC=32, ew_group=4, legalize=True):
    """Build the per-core Bass program.  Parametric for small-config testing."""
    NL = Tc // NQ
    assert NL * NQ == Tc
    NPc = NPAIR * BSc
    NR = NQ * NPc            # partition rows used by the DP
    assert NR <= 128
    W = NQ - 1 + Tc          # wavefront count
    P_M = min(128, Tc)       # matmul m-tile rows
    NMT = (Tc + P_M - 1) // P_M
    NKT = Dc // 128
    assert NKT * 128 == Dc and NMT * P_M == Tc

    nc = bass.Bass("TRN2", target_bir_lowering=False, debug=False)

    tgt_d = nc.dram_tensor("tgt", [BSc, Tc, Dc], F32, kind="ExternalInput")
    oth_d = nc.dram_tensor("oth", [BSc, Tc, Dc], F32, kind="ExternalInput")
    x_d = nc.dram_tensor("x", [BSc, Tc, Dc], F32, kind="ExternalInput")
    rsc_d = nc.dram_tensor("rsc", [128, 1], F32, kind="ExternalInput")
    # partition +NPc shift matrix: psh[k, m] = 1 if m == k + NPc else 0
    psh_d = nc.dram_tensor("pshift", [128, 128], F32, kind="ExternalInput")
    rout_d = nc.dram_tensor("r_raw", [NPc], F32, kind="ExternalOutput")

    with tile.TileContext(nc) as tc:
        with (
            tc.tile_pool(name="inp", bufs=12) as in_pool,
            tc.tile_pool(name="sq", bufs=2) as sq_pool,
            tc.tile_pool(name="stat", bufs=3) as stat_pool,
            tc.tile_pool(name="xh", bufs=12) as xh_pool,
            tc.tile_pool(name="xt", bufs=2 * 3 * NKT) as xt_pool,
            tc.tile_pool(name="psum", bufs=4, space="PSUM") as psum_pool,
            tc.tile_pool(name="esb", bufs=4) as esb_pool,
            tc.tile_pool(name="eskew", bufs=1, space="DRAM") as eskew_pool,
            tc.tile_pool(name="ew", bufs=6) as ew_pool,
            tc.tile_pool(name="dp", bufs=1) as dp_pool,
            tc.tile_pool(name="t1", bufs=3) as t1_pool,
            tc.tile_pool(name="misc", bufs=1) as misc_pool,
        ):
            e_skew = eskew_pool.tile([128, W, NL], F16)

            # --- constants / zero padding for pre-activation waves ---
            zero_t = misc_pool.tile([128, NL], F16, tag="zeros")
            nc.vector.memset(zero_t[:], 0.0)
            for w in range(NQ - 1):
                lo = NPc * (w + 1)
                nc.sync.dma_start(e_skew[lo:128, w, :], zero_t[lo:128, :])
            for k in range(NQ - 1):
                hi = NPc * (k + 1)
                nc.sync.dma_start(e_skew[0:hi, Tc + k, :], zero_t[0:hi, :])
            # rows beyond the DP region (NR..128) are never valid
            if NR < 128:
                for w in range(W):
                    nc.sync.dma_start(e_skew[NR:128, w, :], zero_t[NR:128, :])

            rsc_sb = misc_pool.tile([128, 1], F32, tag="rsc")
            nc.sync.dma_start(rsc_sb[:], rsc_d.ap())
            neg1 = misc_pool.tile([128, 1], F32, tag="neg1")
            nc.vector.memset(neg1[:], -1.0)
            psh_sb = misc_pool.tile([128, 128], F32, tag="psh")
            nc.sync.dma_start(psh_sb[:], psh_d.ap())

            # ---------------- phase 1: grams + exp ----------------
            drams = (tgt_d, oth_d, x_d)
            for b in range(BSc):
                nat = {}
                n2 = stat_pool.tile([128, 16], F32, tag="n2")
                for mi in range(3):
                    for mt in range(NMT):
                        tl = in_pool.tile([P_M, Dc], F32, tag="in")
                        nc.sync.dma_start(
                            tl[:],
                            drams[mi].ap()[b, P_M * mt : P_M * (mt + 1), :],
                        )
                        nat[(mi, mt)] = tl
                        scrap = sq_pool.tile([P_M, Dc], F16, tag="scrap")
                        col = NMT * mi + mt
                        nc.scalar.activation(
                            scrap[:],
                            tl[:],
                            AF.Square,
                            accum_out=n2[:P_M, col : col + 1],
                        )
                ncol = 3 * NMT
                nrm = stat_pool.tile([128, 16], F32, tag="nrm")
                nc.scalar.activation(nrm[:P_M, 0:ncol], n2[:P_M, 0:ncol], AF.Sqrt)
                rinv = stat_pool.tile([128, 16], F32, tag="rinv")
                nc.vector.reciprocal(rinv[:P_M, 0:ncol], nrm[:P_M, 0:ncol])

                # scale rows to unit norm, cast fp16
                xh = {}
                for mi in range(3):
                    for mt in range(NMT):
                        h = xh_pool.tile([P_M, Dc], F16, tag="xh")
                        col = NMT * mi + mt
                        nc.gpsimd.tensor_scalar_mul(
                            h[:], nat[(mi, mt)][:], rinv[:P_M, col : col + 1]
                        )
                        xh[(mi, mt)] = h

                # transpose via DMA xbar: xt[mi][kt] = [128(d), Tc] fp16
                xt = {}
                for mi in range(3):
                    for kt in range(NKT):
                        tr = xt_pool.tile([128, Tc], F16, tag="xt")
                        for mt in range(NMT):
                            nc.sync.dma_start_transpose(
                                tr[:, P_M * mt : P_M * (mt + 1)],
                                xh[(mi, mt)][:, 128 * kt : 128 * (kt + 1)],
                            )
                        xt[(mi, kt)] = tr

                # grams: weights = T̂ᵀ (wg=0) / ÔᵀT (wg=1); rhs = X̂ᵀ and self
                for wg in range(2):
                    for mt in range(NMT):
                        ps_c = psum_pool.tile([P_M, Tc], F32, tag="ps")
                        ps_s = psum_pool.tile([P_M, Tc], F32, tag="ps")
                        for kt in range(NKT):
                            lw = xt[(wg, kt)][:, P_M * mt : P_M * (mt + 1)]
                            nc.tensor.matmul(
                                ps_c[:], lw, xt[(2, kt)][:],
                                start=(kt == 0), stop=(kt == NKT - 1),
                            )
                            nc.tensor.matmul(
                                ps_s[:], lw, xt[(wg, kt)][:],
                                start=(kt == 0), stop=(kt == NKT - 1),
                            )
                        for t_idx, ps in ((2 * wg, ps_c), (2 * wg + 1, ps_s)):
                            esb = esb_pool.tile([P_M, Tc], F16, tag="esb")
                            nc.scalar.activation(
                                esb[:], ps[:], AF.Exp,
                                bias=neg1[:P_M, 0:1], scale=1.0,
                            )
                            p = t_idx * BSc + b
                            for q in range(NQ):
                                nc.sync.dma_start(
                                    e_skew[
                                        NPc * q + p,
                                        P_M * mt + q : P_M * mt + q + P_M,
                                        :,
                                    ],
                                    esb[:, NL * q : NL * (q + 1)],
                                )

            # ---------------- phase 2: soft-DTW wavefront ----------------
            # s buffers: cols 0..NL-1 = lanes of each block (row = NPc*q + p)
            sb = []
            for i in range(2):
                sbt = dp_pool.tile([128, NL], F32, tag=f"s{i}")
                sb.append(sbt)
            nc.vector.memset(sb[0][:], 0.0)
            nc.vector.memset(sb[1][:], 0.0)
            # one-hot t1 for wave 0: virtual S[-1][-1] = 1 feeds lane 0 of q=0
            oh = dp_pool.tile([128, NL], F32, tag="oh")
            nc.vector.memset(oh[:], 0.0)
            nc.vector.memset(oh[0:NPc, 0:1], 1.0)

            # psum tiles holding partition-shifted column tails:
            #   shp[w%3][m] = tails_w[m - NPc]  (zero for m < NPc)
            shp = []
            with tc.tile_pool(name="shps", bufs=1, space="PSUM") as shp_pool:
                for i in range(3):
                    sp = shp_pool.tile([128, 1], F32, tag=f"shp{i}")
                    shp.append(sp)
                # wave 1 reads shp[2] as the (zero) edge of column -1
                nc.vector.memset(shp[2][:, 0:1], 0.0)

                ew_tiles = {}
                for w in range(W):
                    g, r = divmod(w, ew_group)
                    if r == 0:
                        gw = min(ew_group, W - g * ew_group)
                        et = ew_pool.tile([128, ew_group, NL], F16, tag="ew")
                        nc.sync.dma_start(
                            et[:, 0:gw, :],
                            e_skew[:, g * ew_group : g * ew_group + gw, :],
                        )
                        ew_tiles[g] = et
                    cur, prev = sb[w % 2], sb[(w + 1) % 2]
                    if w == 0:
                        t1 = oh
                    else:
                        t1 = t1_pool.tile([128, NL], F32, tag="t1")
                        # t1[l] = prev[l] + prev[l-1]; l=0 uses the shifted
                        # edge (block q-1 tail of the same column) from PSUM
                        nc.vector.tensor_add(
                            t1[:, 1:NL], prev[:, 1:NL], prev[:, 0 : NL - 1]
                        )
                        nc.vector.tensor_add(
                            t1[:, 0:1], prev[:, 0:1], shp[(w - 2) % 3][:, 0:1]
                        )
                    nc.vector.tensor_tensor_scan(
                        cur[:, 0:NL],
                        t1[:],
                        ew_tiles[g][:, r, :],
                        initial=(0.0 if w == 0 else shp[(w - 1) % 3][:, 0:1]),
                        op0=ALU.add,
                        op1=ALU.mult,
                    )
                    if (w + 1) % KC == 0 and w < W - 1:
                        nc.vector.tensor_scalar_mul(
                            cur[:, 0:NL], cur[:, 0:NL], rsc_sb[:, 0:1]
                        )
                        # the edge consumed next wave was produced pre-event
                        nc.vector.tensor_scalar_mul(
                            shp[(w - 1) % 3][:, 0:1],
                            shp[(w - 1) % 3][:, 0:1],
                            rsc_sb[:, 0:1],
                        )
                    # partition-shift the new column tails for init/edge use
                    nc.tensor.matmul(
                        shp[w % 3][:, 0:1],
                        psh_sb[:],
                        cur[:, NL - 1 : NL],
                        start=True,
                        stop=True,
                    )

                final = sb[(W - 1) % 2]
                nc.sync.dma_start(
                    rout_d.ap(), final[NPc * (NQ - 1) : NPc * NQ, NL - 1 : NL]
                )

    if legalize:
        _legalize_waits(nc)
    return nc


def _n_rescale_events(W, KC):
    return sum(1 for w in range(W) if (w + 1) % KC == 0 and w < W - 1)


def make_rsc(KC=32, BSc=BS):
    v = np.empty((128, 1), np.float32)
    NPc = NPAIR * BSc
    for row in range(128):
        p = row % NPc
        c = C_TYPES[p // BSc]
        v[row, 0] = 2.0 ** (-KC * c)
    return v


def make_pshift(BSc=BS):
    NPc = NPAIR * BSc
    v = np.zeros((128, 128), np.float32)
    for k in range(128 - NPc):
        v[k, k + NPc] = 1.0
    return v


_CACHE = {}


def _get_state():
    if "nc" not in _CACHE:
        _CACHE["nc"] = build_nc()
        _CACHE["rsc"] = make_rsc()
        _CACHE["psh"] = make_pshift()
        _CACHE["nev"] = _n_rescale_events(4 - 1 + T, 32)
    return _CACHE


def profile_exec_ns():
    """Run once with NTFF tracing and return max per-core HW exec time (ns).
    Falls back to wall-clock of a bare run if tracing is unavailable."""
    import time

    from concourse.bass_utils import run_bass_kernel_spmd

    st = _get_state()
    rng = np.random.default_rng(0)
    shard = {
        "tgt": rng.standard_normal((BS, T, D)).astype(np.float32),
        "oth": rng.standard_normal((BS, T, D)).astype(np.float32),
        "x": rng.standard_normal((BS, T, D)).astype(np.float32),
        "rsc": st["rsc"],
        "pshift": st["psh"],
    }
    in_maps = [shard] * NCORES
    try:
        res = run_bass_kernel_spmd(
            st["nc"], in_maps, list(range(NCORES)), trace=True
        )
        if res.exec_time_ns:
            return float(res.exec_time_ns)
    except Exception as e:  # noqa: BLE001
        print("profile trace failed:", type(e).__name__, str(e)[:200])
    t0 = time.time()
    run_bass_kernel_spmd(st["nc"], in_maps, list(range(NCORES)))
    return (time.time() - t0) * 1e9


def kernel(TGT, OTH, X, labels):
    from concourse.bass_utils import run_bass_kernel_spmd

    st = _get_state()
    TGT = np.ascontiguousarray(np.asarray(TGT, np.float32))
    OTH = np.ascontiguousarray(np.asarray(OTH, np.float32))
    X = np.ascontiguousarray(np.asarray(X, np.float32))

    in_maps = []
    for c in range(NCORES):
        sl = slice(c * BS, (c + 1) * BS)
        in_maps.append(
            {
                "tgt": TGT[sl],
                "oth": OTH[sl],
                "x": X[sl],
                "rsc": st["rsc"],
                "pshift": st["psh"],
            }
        )
    res = run_bass_kernel_spmd(st["nc"], in_maps, list(range(NCORES)))

    loss = np.empty(B, np.float64)
    ln2 = math.log(2.0)
    for c in range(NCORES):
        r = np.asarray(res.results[c]["r_raw"], np.float64)
        R = np.empty(NP, np.float64)
        for p in range(NP):
            ctype = C_TYPES[p // BS]
            R[p] = -math.log(max(r[p], 1e-300)) - st["nev"] * 32 * ctype * ln2
        for b in range(BS):
            loss[c * BS + b] = (
                R[0 * BS + b]
                - 0.5 * R[1 * BS + b]
                - R[2 * BS + b]
                + 0.5 * R[3 * BS + b]
            )
    return loss.astype(np.float32)
